# revision 1
# baseline (speedup 1.0000x reference)
import numpy as np
import ml_dtypes
import concourse.bass as bass
import concourse.bacc as bacc
import concourse.tile as tile
import concourse.mybir as mybir
from concourse import bass_utils
from contextlib import ExitStack

B = 4
QL = 1024
HIST = 1024
KVL = 2048
H = 4096
NH = 32
D = 128
T = 4096
NCORES = 8
HPC = NH // NCORES          # 4 heads per core
ROPE_BASE = 10000.0
INV_NORM = 1.0 / float(np.sqrt(D))
NEG = -1.0e30

FP = mybir.dt.float32
FPR = mybir.dt.float32r
BF = mybir.dt.bfloat16
F16 = mybir.dt.float16
AX = mybir.AluOpType
AF = mybir.ActivationFunctionType
BF_NP = ml_dtypes.bfloat16


def _build():
    nc = bacc.Bacc("TRN2", num_devices=NCORES)
    xT = nc.dram_tensor("xT", [H, T], BF, kind="ExternalInput")
    w_qk = nc.dram_tensor("w_qk", [H, 2 * HPC * D], BF, kind="ExternalInput")
    w_v = nc.dram_tensor("w_v", [H, HPC * D], BF, kind="ExternalInput")
    wd = nc.dram_tensor("wd", [HPC * D, H], BF, kind="ExternalInput")
    kTh = nc.dram_tensor("kTh", [B, HPC, D, HIST], BF, kind="ExternalInput")
    vhp = nc.dram_tensor("vhp", [B, HPC, 128, HIST], BF, kind="ExternalInput")
    cosT = nc.dram_tensor("cosT", [D, T], FP, kind="ExternalInput")
    sinT = nc.dram_tensor("sinT", [D, T], FP, kind="ExternalInput")
    masksAB = nc.dram_tensor("masksAB", [2, D, 1024], FP, kind="ExternalInput")
    outT = nc.dram_tensor("outT", [H, T], F16, kind="ExternalOutput")

    with tile.TileContext(nc) as tc, ExitStack() as top:
        # ---- constants / resident tensors ----
        cpool = top.enter_context(tc.tile_pool(name="const", bufs=1))
        ones0 = cpool.tile([128, 1], FP)
        nc.vector.memset(ones0[:, :], 1.0)
        ones_col = cpool.tile([128, 1], BF)
        nc.scalar.copy(ones_col[:, :], ones0[:, :])
        ones0r = cpool.tile([1, 128], FP)
        nc.vector.memset(ones0r[:, :], 1.0)
        ones_row = cpool.tile([1, 128], FPR)
        nc.scalar.copy(ones_row[:, :], ones0r[:, :])

        xp = top.enter_context(tc.tile_pool(name="xp", bufs=1))
        wqkp = top.enter_context(tc.tile_pool(name="wqkp", bufs=1))
        qkp = top.enter_context(tc.tile_pool(name="qkp", bufs=1))
        vsp = top.enter_context(tc.tile_pool(name="vsp", bufs=1))
        atp = top.enter_context(tc.tile_pool(name="atp", bufs=1))
        csp = top.enter_context(tc.tile_pool(name="csp", bufs=1))

        wqk_sb = []
        x_t = [None] * 32
        cs_t = [None, None]

        def load_x(b):
            c0 = b * QL
            for k in range(32):
                xt = xp.tile([128, QL], BF, name=f"x{k}")
                nc.sync.dma_start(out=xt[:, :], in_=xT[k * 128:(k + 1) * 128, c0:c0 + QL])
                x_t[k] = xt
            ct = csp.tile([128, QL], FP, name="cos")
            nc.sync.dma_start(out=ct[:, :], in_=cosT[:, c0:c0 + QL])
            st = csp.tile([128, QL], FP, name="sin")
            nc.sync.dma_start(out=st[:, :], in_=sinT[:, c0:c0 + QL])
            cs_t[0], cs_t[1] = ct, st

        # initial loads: interleave x(0) and w_qk so PE can start ASAP
        c0 = 0
        for k in range(32):
            xt = xp.tile([128, QL], BF, name=f"x{k}")
            nc.sync.dma_start(out=xt[:, :], in_=xT[k * 128:(k + 1) * 128, 0:QL])
            x_t[k] = xt
            wt = wqkp.tile([128, 2 * HPC * D], BF, name=f"wqk{k}")
            nc.sync.dma_start(out=wt[:, :], in_=w_qk[k * 128:(k + 1) * 128, :])
            wqk_sb.append(wt)
        ct = csp.tile([128, QL], FP, name="cos")
        nc.sync.dma_start(out=ct[:, :], in_=cosT[:, 0:QL])
        st = csp.tile([128, QL], FP, name="sin")
        nc.sync.dma_start(out=st[:, :], in_=sinT[:, 0:QL])
        cs_t[0], cs_t[1] = ct, st

        for b in range(B):
            c0 = b * QL
            qk_t = []
            v_sb = []

            # ================= stage A1: q^T,k^T with RoPE =================
            with ExitStack() as actx:
                psA1 = actx.enter_context(tc.tile_pool(name=f"psA1_{b}", bufs=1, space="PSUM"))
                psA2 = actx.enter_context(tc.tile_pool(name=f"psA2_{b}", bufs=1, space="PSUM"))
                rotp = actx.enter_context(tc.tile_pool(name=f"rot{b}", bufs=2))
                tmpp = actx.enter_context(tc.tile_pool(name=f"tmp{b}", bufs=2))
                for m in range(8):
                    pst = psA1.tile([128, QL], FP, name="pst", bufs=2)
                    for k in range(32):
                        lw = wqk_sb[k][:, m * 128:(m + 1) * 128]
                        for ns in range(2):
                            nc.tensor.matmul(
                                pst[:, ns * 512:(ns + 1) * 512], lw,
                                x_t[k][:, ns * 512:(ns + 1) * 512],
                                start=(k == 0), stop=(k == 31))
                    rot = rotp.tile([128, QL], FP, name="rot")
                    nc.scalar.mul(rot[0:64, :], pst[64:128, :], -1.0)
                    nc.scalar.copy(rot[64:128, :], pst[0:64, :])
                    tmp = tmpp.tile([128, QL], FP, name="tmp")
                    nc.vector.tensor_tensor(
                        out=tmp[:, :], in0=pst[:, :], in1=cs_t[0][:, :], op=AX.mult)
                    nc.vector.tensor_tensor(
                        out=rot[:, :], in0=rot[:, :], in1=cs_t[1][:, :], op=AX.mult)
                    qk = qkp.tile([128, QL], BF, name=f"qk{m}")
                    nc.vector.tensor_tensor(
                        out=qk[:, :], in0=tmp[:, :], in1=rot[:, :], op=AX.add)
                    qk_t.append(qk)

                # ====== stage A2: new V (t on partitions), shares PSUM ctx ======
                wvp = actx.enter_context(tc.tile_pool(name=f"wvp{b}", bufs=6))
                for hf in range(2):
                    psv = [psA2.tile([128, 512], FP, name=f"psv{t}", bufs=1)
                           for t in range(4)]
                    for k in range(32):
                        wvt = wvp.tile([128, 512], BF, name="wv")
                        nc.sync.dma_start(out=wvt[:, :], in_=w_v[k * 128:(k + 1) * 128, :])
                        for t4 in range(4):
                            t = hf * 4 + t4
                            nc.tensor.matmul(
                                psv[t4][:, :], x_t[k][:, t * 128:(t + 1) * 128], wvt[:, :],
                                start=(k == 0), stop=(k == 31))
                    for t4 in range(4):
                        t = hf * 4 + t4
                        vt = vsp.tile([128, 512], BF, name=f"v{t}")
                        if t4 % 2 == 0:
                            nc.scalar.copy(vt[:, :], psv[t4][:, :])
                        else:
                            nc.vector.tensor_scalar_mul(vt[:, :], psv[t4][:, :], 1.0)
                        v_sb.append(vt)

            # ================= stage B: attention =================
            at_t = [[None, None] for _ in range(HPC)]
            with ExitStack() as bctx:
                psSG = bctx.enter_context(tc.tile_pool(name=f"psSG{b}", bufs=1, space="PSUM"))
                psAT = bctx.enter_context(tc.tile_pool(name=f"psAT{b}", bufs=1, space="PSUM"))
                psDN = bctx.enter_context(tc.tile_pool(name=f"psDN{b}", bufs=1, space="PSUM"))
                psBC = bctx.enter_context(tc.tile_pool(name=f"psBC{b}", bufs=1, space="PSUM"))
                khp = bctx.enter_context(tc.tile_pool(name=f"khp{b}", bufs=2))
                vhp_p = bctx.enter_context(tc.tile_pool(name=f"vhp{b}", bufs=2))
                pp = bctx.enter_context(tc.tile_pool(name=f"pp{b}", bufs=3))
                pap = bctx.enter_context(tc.tile_pool(name=f"pap{b}", bufs=2))
                smp = bctx.enter_context(tc.tile_pool(name=f"smp{b}", bufs=1))
                bcp = bctx.enter_context(tc.tile_pool(name=f"bcp{b}", bufs=1))
                mkp = bctx.enter_context(tc.tile_pool(name=f"mkp{b}", bufs=1))
                sxp = bctx.enter_context(tc.tile_pool(name=f"sxp{b}", bufs=1))

                mask_t = []
                for d in range(2):
                    mt = mkp.tile([128, 1024], FP, name=f"mask{d}")
                    nc.sync.dma_start(out=mt[:, :], in_=masksAB[d, :, :])
                    mask_t.append(mt)

                pend = [None]          # deferred tail of previous unit

                def flush_tail():
                    if pend[0] is None:
                        return
                    attn_ps, den_sb_rec, h2, qc2 = pend[0]
                    rec = den_sb_rec
                    bc_ps = psBC.tile([128, 512], FP, name="bc", bufs=1)
                    nc.tensor.matmul(bc_ps[:, :], ones_row[:, :], rec[:, :],
                                     start=True, stop=True)
                    bc_sb = bcp.tile([128, 512], FP, name="bcsb")
                    nc.scalar.copy(bc_sb[:, :], bc_ps[:, :])
                    at = atp.tile([128, 512], BF, name=f"at{h2}_{qc2}")
                    nc.vector.tensor_tensor(
                        out=at[:, :], in0=attn_ps[:, :], in1=bc_sb[:, :], op=AX.mult)
                    at_t[h2][qc2] = at
                    pend[0] = None

                for h in range(HPC):
                    kh_t = khp.tile([128, HIST], BF, name="kh")
                    nc.sync.dma_start(out=kh_t[:, :], in_=kTh[b, h, :, :])
                    vh_t = vhp_p.tile([128, HIST], BF, name="vh")
                    nc.sync.dma_start(out=vh_t[:, :], in_=vhp[b, h, :, :])

                    def k_src(ti):
                        if ti < 8:
                            return kh_t[:, ti * 128:(ti + 1) * 128]
                        return qk_t[4 + h][:, (ti - 8) * 128:(ti - 7) * 128]

                    def v_src(ti):
                        if ti < 8:
                            return vh_t[:, ti * 128:(ti + 1) * 128]
                        return v_sb[ti - 8][:, h * 128:(h + 1) * 128]

                    for qc in range(2):
                        n_kv = 12 + 4 * qc
                        n_g = n_kv // 2
                        q_ap = qk_t[h][:, qc * 512:(qc + 1) * 512]
                        attn_ps = psAT.tile([128, 512], FP, name="attn", bufs=2)
                        den_ps = psDN.tile([1, 512], FP, name="den", bufs=1)
                        p_acc = pap.tile([128, 1024], BF, name="pacc")
                        p_list = [None] * n_g

                        def emit_da(g):
                            for j in range(2):
                                ti = 2 * g + j
                                psl = p_list[g][:, j * 512:(j + 1) * 512]
                                nc.tensor.matmul(attn_ps[:, :], v_src(ti), psl,
                                                 start=(ti == 0), stop=(ti == n_kv - 1))

                        for g in range(n_g):
                            sg = psSG.tile([128, 1024], FP, name="sg", bufs=2)
                            for j in range(2):
                                nc.tensor.matmul(
                                    sg[:, j * 512:(j + 1) * 512], k_src(2 * g + j), q_ap,
                                    start=True, stop=True)
                            di = g - (n_g - 2)
                            p = pp.tile([128, 1024], BF, name="p")
                            if di >= 0:
                                s_sb = sxp.tile([128, 1024], FP, name="sx")
                                nc.vector.tensor_tensor(
                                    out=s_sb[:, :], in0=sg[:, :], in1=mask_t[di][:, :],
                                    op=AX.add)
                                nc.scalar.activation(p[:, :], s_sb[:, :], AF.Exp,
                                                     scale=INV_NORM)
                            else:
                                nc.scalar.activation(p[:, :], sg[:, :], AF.Exp,
                                                     scale=INV_NORM)
                            p_list[g] = p
                            if g == 0:
                                nc.vector.tensor_scalar_mul(p_acc[:, :], p[:, :], 1.0)
                            else:
                                nc.vector.tensor_tensor(
                                    out=p_acc[:, :], in0=p_acc[:, :], in1=p[:, :],
                                    op=AX.add)
                            if g == 1:
                                flush_tail()
                            if g >= 2:
                                emit_da(g - 2)
                        emit_da(n_g - 2)
                        emit_da(n_g - 1)
                        for j in range(2):
                            nc.tensor.matmul(
                                den_ps[:, :], ones_col[:, :],
                                p_acc[:, j * 512:(j + 1) * 512],
                                start=(j == 0), stop=(j == 1))
                        den_sb = smp.tile([1, 512], FP, name="densb")
                        nc.scalar.copy(den_sb[:, :], den_ps[:, :])
                        rec = smp.tile([1, 512], FPR, name="rec")
                        with nc.allow_low_precision(reason="fp32r bits are fp32"):
                            nc.vector.reciprocal(rec[:, :], den_sb[:, :])
                        pend[0] = (attn_ps, rec, h, qc)
                flush_tail()
                # prefetch next seq's activations behind B's own DMAs
                if b + 1 < B:
                    load_x(b + 1)

            # ================= stage C: dense partial out =================
            with ExitStack() as cctx:
                psC = cctx.enter_context(tc.tile_pool(name=f"psC{b}", bufs=1, space="PSUM"))
                wdp = cctx.enter_context(tc.tile_pool(name=f"wdp{b}", bufs=1))
                op_ = cctx.enter_context(tc.tile_pool(name=f"op{b}", bufs=3))
                wd_sb = []
                for k in range(HPC):
                    wt = wdp.tile([128, H], BF, name=f"wd{k}")
                    nc.sync.dma_start(out=wt[:, :], in_=wd[k * 128:(k + 1) * 128, :])
                    wd_sb.append(wt)
                for m in range(32):
                    pso = psC.tile([128, 1024], FP, name="pso", bufs=2)
                    for k in range(HPC):
                        for qc in range(2):
                            nc.tensor.matmul(
                                pso[:, qc * 512:(qc + 1) * 512],
                                wd_sb[k][:, m * 128:(m + 1) * 128],
                                at_t[k][qc][:, :],
                                start=(k == 0), stop=(k == HPC - 1))
                    o = op_.tile([128, 1024], F16, name="o")
                    nc.scalar.copy(o[:, 0:512], pso[:, 0:512])
                    nc.vector.tensor_scalar_mul(o[:, 512:1024], pso[:, 512:1024], 1.0)
                    nc.sync.dma_start(
                        out=outT[m * 128:(m + 1) * 128, c0:c0 + QL], in_=o[:, :])

    nc.compile()
    return nc


_NC = None
_LAST_EXEC_NS = None


def _host_prep(hidden_states, w_qkv, w_dense, past_key, past_value,
               block_offsets, position_ids_1d):
    xT = np.ascontiguousarray(np.asarray(hidden_states, np.float32)[0].T).astype(BF_NP)
    w_qkv = np.asarray(w_qkv, np.float32)
    w_dense = np.asarray(w_dense, np.float32)
    bo = np.asarray(block_offsets)
    pos = np.asarray(position_ids_1d)

    inv_freq = (1.0 / (ROPE_BASE ** (np.arange(0, D, 2, dtype=np.float32) / D))).astype(np.float32)
    f2 = np.concatenate([inv_freq, inv_freq]).astype(np.float32)
    ang = pos.astype(np.float32)[None, :] * f2[:, None]          # [128, T]
    cosT = np.cos(ang).astype(np.float32)
    sinT = np.sin(ang).astype(np.float32)

    i = np.arange(128)[:, None]
    j = np.arange(512)[None, :]
    m4 = [np.where(i + 128 * d <= j, np.float32(0.0), np.float32(NEG)) for d in range(4)]
    masksAB = np.stack([
        np.concatenate([m4[0], m4[1]], axis=1),
        np.concatenate([m4[2], m4[3]], axis=1),
    ]).astype(np.float32)                                        # [2, 128, 1024]

    nhb = HIST // 64                                             # history blocks per seq
    hist_k = np.asarray(past_key)[bo[:, :nhb]].reshape(B, HIST, NH, D)
    hist_v = np.asarray(past_value)[bo[:, :nhb]].reshape(B, HIST, NH, D)
    # [B, NH, D, HIST] (d-major keys)
    kTh_all = hist_k.transpose(0, 2, 3, 1).astype(BF_NP)
    # [B, NH, 128, 8*128]: vhp[b,h,p,c*128+d] = hist_v[b, c*128+p, h, d]
    vhp_all = hist_v.reshape(B, 8, 128, NH, D).transpose(0, 3, 2, 1, 4) \
        .reshape(B, NH, 128, HIST).astype(BF_NP)

    wq = w_qkv.reshape(H, NH, 3, D)
    in_maps = []
    for c in range(NCORES):
        hs = slice(c * HPC, (c + 1) * HPC)
        w_qk_c = np.concatenate(
            [wq[:, hs, 0, :].reshape(H, HPC * D), wq[:, hs, 1, :].reshape(H, HPC * D)],
            axis=1)
        in_maps.append({
            "xT": xT,
            "w_qk": np.ascontiguousarray(w_qk_c).astype(BF_NP),
            "w_v": np.ascontiguousarray(wq[:, hs, 2, :].reshape(H, HPC * D)).astype(BF_NP),
            "wd": np.ascontiguousarray(w_dense[c * HPC * D:(c + 1) * HPC * D, :]).astype(BF_NP),
            "kTh": np.ascontiguousarray(kTh_all[:, hs]),
            "vhp": np.ascontiguousarray(vhp_all[:, hs]),
            "cosT": cosT,
            "sinT": sinT,
            "masksAB": masksAB,
        })
    return in_maps


def kernel(hidden_states, w_qkv, w_dense, past_key, past_value,
           block_offsets, position_ids_1d):
    global _NC, _LAST_EXEC_NS
    if _NC is None:
        _NC = _build()
    in_maps = _host_prep(hidden_states, w_qkv, w_dense, past_key, past_value,
                         block_offsets, position_ids_1d)
    res = bass_utils.run_bass_kernel_spmd(_NC, in_maps, core_ids=list(range(NCORES)))
    _LAST_EXEC_NS = getattr(res, "exec_time_ns", None)
    acc = np.zeros((H, T), np.float32)
    for c in range(NCORES):
        acc += np.asarray(res.results[c]["outT"], dtype=np.float32)
    return np.ascontiguousarray(acc.T).reshape(1, T, H).astype(np.float32)

